# revision 6
# baseline (speedup 1.0000x reference)
"""Trainium2 Bass kernel for nn_AnatomicalTextEnhancer (retrieval_knn).

Data-parallel over the phrase axis N: each of the 8 NeuronCores scores
all 29 regions x 32 batch queries against its 1024-phrase shard of the
text table; the host combines the 8 per-core (top-sim, top-idx) pairs.

Numerics: every tensor entering the TensorEngine is rounded once to
fp32r (RNE at the 11-bit-mantissa boundary, matching the HW DMA cast);
all downstream matmul arithmetic is then exact (f32 PSUM accumulation),
so the device computes the exact similarity of a once-perturbed dataset.
This was verified host-side to reproduce the f32 reference argmax with
zero flips and best-sim rel-err ~6e-5.
"""

import os
import sys
import tempfile

import numpy as np

sys.path.insert(0, "/opt/trn_rl_repo")
sys.path.insert(0, "/opt/pypackages")

import concourse.bass as bass
import concourse.mybir as mybir
import concourse.tile as tile
from concourse import bacc
from concourse.bass_utils import run_bass_kernel_spmd

F32 = mybir.dt.float32
F32R = mybir.dt.float32r
U32 = mybir.dt.uint32

R, B, H, P = 29, 32, 768, 512
NH = H // 128  # 6 h-chunks
NP = P // 128  # 4 p-chunks
NCORES = 8
MASKVAL = -float(2**30)


def rne12(x):
    """Round f32 -> fp32r (11-bit explicit mantissa), RNE. Matches TRN2."""
    b = np.ascontiguousarray(np.asarray(x, np.float32)).view(np.uint32)
    lsb = (b >> np.uint32(12)) & np.uint32(1)
    return ((b + np.uint32(0x7FF) + lsb) & np.uint32(0xFFFFF000)).view(np.float32)


def build(nshard=1024, nregions=R, nbatch=B):
    """Build the per-core SPMD graph. All cores run the same NEFF."""
    NT = nshard // 512  # 512-wide n-tiles per region
    BRC = (nregions * nbatch + 127) // 128  # 128-wide br chunks (br = r*32+b)
    BRW = BRC * 128

    nc = bacc.Bacc("TRN2", target_bir_lowering=False, debug=False)

    emb = nc.dram_tensor("emb", [nregions, H, nshard], F32R, kind="ExternalInput").ap()
    wt = nc.dram_tensor("wt", [H, P], F32R, kind="ExternalInput").ap()
    wtT = nc.dram_tensor("wtT", [P, H], F32R, kind="ExternalInput").ap()
    wv = nc.dram_tensor("wv", [H, P], F32R, kind="ExternalInput").ap()
    rfT = nc.dram_tensor("rfT", [H, BRW], F32R, kind="ExternalInput").ap()
    bvrow = nc.dram_tensor("bvrow", [1, P], F32R, kind="ExternalInput").ap()
    btp = nc.dram_tensor("btp", [128, NP], F32, kind="ExternalInput").ap()
    btc = nc.dram_tensor("btc", [128, NP], F32R, kind="ExternalInput").ap()
    onesf_d = nc.dram_tensor("onesf", [128, 128], F32R, kind="ExternalInput").ap()
    onesr_d = nc.dram_tensor("onesr", [1, 512], F32R, kind="ExternalInput").ap()
    identf_d = nc.dram_tensor("identf", [128, 128], F32R, kind="ExternalInput").ap()
    mask_d = nc.dram_tensor(
        "maskrow", [nregions, nshard], F32R, kind="ExternalInput"
    ).ap()

    out_sim = nc.dram_tensor("out_sim", [nbatch, nregions], F32, kind="ExternalOutput").ap()
    out_idx = nc.dram_tensor("out_idx", [nbatch, nregions], U32, kind="ExternalOutput").ap()

    with tile.TileContext(nc) as tc:
        with (
            tc.tile_pool(name="const", bufs=1) as constp,
            tc.tile_pool(name="wpool", bufs=1) as wpool,
        ):
            # ---------- constants / weights ----------
            wt_sb = [constp.tile([128, P], F32R, tag=f"wt{h}", name=f"wt_sb{h}") for h in range(NH)]
            for h in range(NH):
                nc.sync.dma_start(wt_sb[h][:], wt[h * 128 : (h + 1) * 128, :])
            btp_sb = constp.tile([128, NP], F32, tag="btp")
            nc.sync.dma_start(btp_sb[:], btp)
            onesf = constp.tile([128, 128], F32R, tag="onesf")
            nc.sync.dma_start(onesf[:], onesf_d)
            onesr = constp.tile([1, 512], F32R, tag="onesr")
            nc.sync.dma_start(onesr[:], onesr_d)


            # ---------- visual path ----------
            w_sb = [wpool.tile([128, BRW], F32R, tag=f"w{h}", name=f"w_sb{h}") for h in range(NH)]
            c_sb = wpool.tile([1, BRW], F32R, tag="c")
            with (
                tc.tile_pool(name="vps", bufs=2, space="PSUM") as vps,
                tc.tile_pool(name="vsb", bufs=2) as vsb,
                tc.tile_pool(name="vconst", bufs=1) as vconst,
                tc.tile_pool(name="vtps", bufs=2, space="PSUM") as vtps,
            ):
                wtT_sb = [vconst.tile([128, H], F32R, tag=f"wtT{p}", name=f"wtT_sb{p}") for p in range(NP)]
                for p in range(NP):
                    nc.sync.dma_start(wtT_sb[p][:], wtT[p * 128 : (p + 1) * 128, :])
                wv_sb = [vconst.tile([128, P], F32R, tag=f"wv{h}", name=f"wv_sb{h}") for h in range(NH)]
                for h in range(NH):
                    nc.sync.dma_start(wv_sb[h][:], wv[h * 128 : (h + 1) * 128, :])
                rfT_sb = [vconst.tile([128, BRW], F32R, tag=f"rfT{h}", name=f"rfT_sb{h}") for h in range(NH)]
                for h in range(NH):
                    nc.sync.dma_start(rfT_sb[h][:], rfT[h * 128 : (h + 1) * 128, :])
                bv_sb = vconst.tile([1, P], F32R, tag="bv")
                nc.sync.dma_start(bv_sb[:], bvrow)
                btc_sb = vconst.tile([128, NP], F32R, tag="btc")
                nc.sync.dma_start(btc_sb[:], btc)
                identf = vconst.tile([128, 128], F32R, tag="identf")
                nc.sync.dma_start(identf[:], identf_d)
                vn_sb = []
                for bc in range(BRC):
                    mv_ps = vps.tile([128, P], F32, tag="mv")
                    for h in range(NH):
                        nc.tensor.matmul(
                            mv_ps[:],
                            rfT_sb[h][:, bc * 128 : (bc + 1) * 128],
                            wv_sb[h][:],
                            start=(h == 0),
                            stop=False,
                        )
                    nc.tensor.matmul(
                        mv_ps[:],
                        onesr[0:1, 0:128],
                        bv_sb[:],
                        start=False,
                        stop=True,
                    )
                    # qv = sum_p mv^2 per br
                    sq_scratch = vsb.tile([128, P], F32, tag="sqs")
                    qv = vsb.tile([128, 1], F32, tag="qv")
                    nc.scalar.activation(
                        sq_scratch[:],
                        mv_ps[:],
                        mybir.ActivationFunctionType.Square,
                        bias=0.0,
                        scale=1.0,
                        accum_out=qv[:],
                    )
                    # rv = rsqrt(qv) with one Newton step
                    qi = vsb.tile([128, 1], F32, tag="qi")
                    nc.vector.reciprocal(qi[:], qv[:])
                    r0 = vsb.tile([128, 1], F32, tag="r0")
                    nc.scalar.sqrt(r0[:], qi[:])
                    t1 = vsb.tile([128, 1], F32, tag="t1")
                    nc.vector.tensor_mul(t1[:], r0[:], r0[:])
                    nc.vector.tensor_mul(t1[:], t1[:], qv[:])
                    nc.scalar.activation(
                        t1[:],
                        t1[:],
                        mybir.ActivationFunctionType.Copy,
                        bias=1.5,
                        scale=-0.5,
                    )
                    rv = vsb.tile([128, 1], F32, tag="rv")
                    nc.vector.tensor_mul(rv[:], r0[:], t1[:])
                    # vn = mv * rv  (fp32r out)
                    vn_t = vsb.tile([128, P], F32R, tag="vn")
                    nc.scalar.activation(
                        vn_t[:],
                        mv_ps[:],
                        mybir.ActivationFunctionType.Copy,
                        bias=0.0,
                        scale=rv[:],
                    )
                    vn_sb.append((bc, vn_t))

                # vnT via PE transpose
                vnT_sb = [vconst.tile([128, BRW], F32R, tag=f"vnT{p}", name=f"vnT_sb{p}") for p in range(NP)]
                for bc, vn_t in vn_sb:
                    for p in range(NP):
                        tp = vtps.tile([128, 128], F32R, tag="vt")
                        nc.tensor.transpose(
                            tp[:], vn_t[:, p * 128 : (p + 1) * 128], identf[:]
                        )
                        nc.scalar.copy(
                            vnT_sb[p][:, bc * 128 : (bc + 1) * 128],
                            tp[:].bitcast(F32),
                        )

                # w[h, br] = sum_p WtT[p,h] * vnT[p,br]  (fp32r out)
                spans = [(o, min(512, BRW - o)) for o in range(0, BRW, 512)]
                for h in range(NH):
                    for off, wdt in spans:
                        w_ps = vps.tile([128, 512], F32, tag="wps")
                        for p in range(NP):
                            nc.tensor.matmul(
                                w_ps[:, 0:wdt],
                                wtT_sb[p][:, h * 128 : (h + 1) * 128],
                                vnT_sb[p][:, off : off + wdt],
                                start=(p == 0),
                                stop=(p == NP - 1),
                            )
                        nc.scalar.copy(
                            w_sb[h][:, off : off + wdt],
                            w_ps[:, 0:wdt].bitcast(F32),
                        )
                # c[br] = sum_p btc[p] * vnT[p, br]
                for off, wdt in spans:
                    c_ps = vps.tile([1, 512], F32, tag="cps")
                    for p in range(NP):
                        nc.tensor.matmul(
                            c_ps[:, 0:wdt],
                            btc_sb[:, p : p + 1],
                            vnT_sb[p][:, off : off + wdt],
                            start=(p == 0),
                            stop=(p == NP - 1),
                        )
                    nc.scalar.copy(
                        c_sb[0:1, off : off + wdt],
                        c_ps[:, 0:wdt].bitcast(F32),
                    )

            # ---------- main loop over regions ----------
            outsim_sb = wpool.tile([nbatch, nregions], F32, tag="osim")
            outidx_sb = wpool.tile([nbatch, nregions], U32, tag="oidx")
            with (
                tc.tile_pool(name="embp", bufs=2 * NH * NT) as embp,
                tc.tile_pool(name="zps", bufs=4, space="PSUM") as zps,
                tc.tile_pool(name="zsqp", bufs=2 * NP * NT) as zsqp,
                tc.tile_pool(name="qps", bufs=2, space="PSUM") as qps,
                tc.tile_pool(name="nps", bufs=2, space="PSUM") as nps,
                tc.tile_pool(name="spool", bufs=2) as spool,
                tc.tile_pool(name="qsb", bufs=2 * NT) as qsbp,
            ):
                for r in range(nregions):
                    m_t = spool.tile([1, nshard], F32R, tag="mrow", name=f"m_{r}")
                    nc.sync.dma_start(m_t[:], mask_d[r : r + 1, :])
                    embT = {}
                    for h in range(NH):
                        for t in range(NT):
                            e_t = embp.tile([128, 512], F32R, tag="emb")
                            nc.sync.dma_start(
                                e_t[:],
                                emb[r, h * 128 : (h + 1) * 128, t * 512 : (t + 1) * 512],
                            )
                            embT[h, t] = e_t

                    # z + zsq
                    zsq = {}
                    for p in range(NP):
                        zt = [zps.tile([128, 512], F32, tag="z", name=f"z_{r}_{p}_{_t}") for _t in range(NT)]
                        for h in range(NH):
                            for t in range(NT):
                                nc.tensor.matmul(
                                    zt[t][:],
                                    wt_sb[h][:, p * 128 : (p + 1) * 128],
                                    embT[h, t][:],
                                    start=(h == 0),
                                    stop=(h == NH - 1),
                                )
                        for t in range(NT):
                            zq = zsqp.tile([128, 512], F32R, tag="zsq")
                            nc.scalar.activation(
                                zq[:],
                                zt[t][:],
                                mybir.ActivationFunctionType.Square,
                                bias=btp_sb[:, p : p + 1],
                                scale=1.0,
                            )
                            zsq[p, t] = zq

                    # q-reduction + rsqrt chain (rows 0..31 used)
                    rsq = []
                    for t in range(NT):
                        q_ps = qps.tile([128, 512], F32, tag="q")
                        for p in range(NP):
                            nc.tensor.matmul(
                                q_ps[:],
                                onesf[:],
                                zsq[p, t][:],
                                start=(p == 0),
                                stop=(p == NP - 1),
                            )
                        qi = qsbp.tile([nbatch, 512], F32, tag="qi")
                        nc.vector.reciprocal(qi[:], q_ps[0:nbatch, :])
                        r0 = qsbp.tile([nbatch, 512], F32, tag="r0")
                        nc.scalar.sqrt(r0[:], qi[:])
                        # Newton: r1 = r0*(1.5 - 0.5*q*r0^2)
                        t1 = qsbp.tile([nbatch, 512], F32, tag="t1")
                        nc.vector.tensor_mul(t1[:], r0[:], r0[:])
                        nc.vector.tensor_mul(t1[:], t1[:], q_ps[0:nbatch, :])
                        nc.scalar.activation(
                            t1[:],
                            t1[:],
                            mybir.ActivationFunctionType.Copy,
                            bias=1.5,
                            scale=-0.5,
                        )
                        r1 = qsbp.tile([nbatch, 512], F32, tag="r1")
                        nc.vector.tensor_mul(r1[:], r0[:], t1[:])
                        rsq.append(r1)

                    # num + sims
                    sims = spool.tile([nbatch, nshard], F32, tag="sims")
                    for t in range(NT):
                        n_ps = nps.tile([nbatch, 512], F32, tag="num")
                        for h in range(NH):
                            nc.tensor.matmul(
                                n_ps[:],
                                w_sb[h][:, r * nbatch : (r + 1) * nbatch],
                                embT[h, t][:],
                                start=(h == 0),
                                stop=False,
                            )
                        nc.tensor.matmul(
                            n_ps[:],
                            c_sb[0:1, r * nbatch : (r + 1) * nbatch],
                            onesr[:],
                            start=False,
                            stop=False,
                        )
                        nc.tensor.matmul(
                            n_ps[:],
                            onesr[0:1, 0:nbatch],
                            m_t[0:1, t * 512 : (t + 1) * 512],
                            start=False,
                            stop=True,
                        )
                        nc.vector.tensor_mul(
                            sims[:, t * 512 : (t + 1) * 512], n_ps[:], rsq[t][:]
                        )

                    # top-1
                    top8 = spool.tile([nbatch, 8], F32, tag="top8")
                    idx8 = spool.tile([nbatch, 8], U32, tag="idx8")
                    nc.vector.max(top8[:], sims[:])
                    nc.vector.max_index(idx8[:], top8[:], sims[:])
                    nc.scalar.copy(outsim_sb[:, r : r + 1], top8[:, 0:1])
                    nc.vector.tensor_copy(outidx_sb[:, r : r + 1], idx8[:, 0:1])

                nc.sync.dma_start(out_sim, outsim_sb[:])
                nc.sync.dma_start(out_idx, outidx_sb[:])

    nc.compile()
    return nc


_CACHE = {}


def _get_compiled(nshard, nregions, nbatch):
    key = (nshard, nregions, nbatch)
    if key not in _CACHE:
        _CACHE[key] = build(nshard, nregions, nbatch)
    return _CACHE[key]


def prep_inputs(visual_features, text_embeddings, Wv, bv, Wt, bt, valid_counts,
                nshard=1024):
    """Host-side layout prep + fp32r pre-rounding. Returns per-core in_maps."""
    vf = np.asarray(visual_features, np.float32)
    te = np.asarray(text_embeddings, np.float32)
    Wv = np.asarray(Wv, np.float32)
    bv = np.asarray(bv, np.float32)
    Wt = np.asarray(Wt, np.float32)
    bt = np.asarray(bt, np.float32)
    vc = np.asarray(valid_counts, np.int64)

    nregions, N = te.shape[0], te.shape[1]
    nbatch = vf.shape[0]
    BRC = (nregions * nbatch + 127) // 128
    BRW = BRC * 128

    rf = vf[:, 1 : 1 + nregions, :]  # [B, R, H]
    rfT = np.ones((H, BRW), np.float32)
    rfT[:, : nregions * nbatch] = rf.transpose(2, 1, 0).reshape(H, -1)

    btp = np.ascontiguousarray(bt.reshape(NP, 128).T)

    common = {
        "wt": rne12(Wt),
        "wtT": rne12(np.ascontiguousarray(Wt.T)),
        "wv": rne12(Wv),
        "rfT": rne12(rfT),
        "bvrow": rne12(bv[None, :]),
        "btp": btp,
        "btc": rne12(btp),
        "onesf": np.ones((128, 128), np.float32),
        "onesr": np.ones((1, 512), np.float32),
        "identf": np.eye(128, dtype=np.float32),
    }

    teT = te.transpose(0, 2, 1)  # [R, H, N] view
    in_maps = []
    for c in range(NCORES):
        sl = slice(c * nshard, (c + 1) * nshard)
        m = dict(common)
        m["emb"] = rne12(teT[:, :, sl])
        nidx = np.arange(c * nshard, (c + 1) * nshard)
        mask = np.where(nidx[None, :] < vc[:, None], 0.0, MASKVAL).astype(np.float32)
        m["maskrow"] = mask
        in_maps.append(m)
    return in_maps


def combine(results, nshard=1024):
    """Host-side combine of per-core partial results."""
    sims8 = np.stack([r["out_sim"] for r in results])  # [8, B, R]
    idx8 = np.stack([r["out_idx"] for r in results]).astype(np.int64)
    gc = np.argmax(sims8, axis=0)  # [B, R]; first max wins ties
    best_sim = np.take_along_axis(sims8, gc[None], 0)[0]
    local = np.take_along_axis(idx8, gc[None], 0)[0]
    best_idx = (local + nshard * gc).astype(np.int32)
    return best_sim.astype(np.float32), best_idx


def kernel(visual_features, text_embeddings, Wv, bv, Wt, bt, valid_counts):
    nshard = text_embeddings.shape[1] // NCORES
    in_maps = prep_inputs(
        visual_features, text_embeddings, Wv, bv, Wt, bt, valid_counts, nshard
    )
    nc = _get_compiled(nshard, text_embeddings.shape[0], visual_features.shape[0])
    res = run_bass_kernel_spmd(nc, in_maps, core_ids=list(range(NCORES)))
    return combine(res.results, nshard)


# revision 7
# speedup vs baseline: 1.0651x; 1.0651x over previous
"""Trainium2 Bass kernel for nn_AnatomicalTextEnhancer (retrieval_knn).

Data-parallel over the phrase axis N: each of the 8 NeuronCores scores
all 29 regions x 32 batch queries against its 1024-phrase shard of the
text table; the host combines the 8 per-core (top-sim, top-idx) pairs.

Numerics: every tensor entering the TensorEngine is rounded once to
fp32r (RNE at the 11-bit-mantissa boundary, matching the HW DMA cast);
all downstream matmul arithmetic is then exact (f32 PSUM accumulation),
so the device computes the exact similarity of a once-perturbed dataset.
This was verified host-side to reproduce the f32 reference argmax with
zero flips and best-sim rel-err ~6e-5.
"""

import os
import sys
import tempfile

import numpy as np

sys.path.insert(0, "/opt/trn_rl_repo")
sys.path.insert(0, "/opt/pypackages")

import concourse.bass as bass
import concourse.mybir as mybir
import concourse.tile as tile
from concourse import bacc
from concourse.bass_utils import run_bass_kernel_spmd

F32 = mybir.dt.float32
F32R = mybir.dt.float32r
U32 = mybir.dt.uint32

R, B, H, P = 29, 32, 768, 512
NH = H // 128  # 6 h-chunks
NP = P // 128  # 4 p-chunks
NCORES = 8
MASKVAL = -float(2**30)


def rne12(x):
    """Round f32 -> fp32r (11-bit explicit mantissa), RNE. Matches TRN2."""
    b = np.ascontiguousarray(np.asarray(x, np.float32)).view(np.uint32)
    lsb = (b >> np.uint32(12)) & np.uint32(1)
    return ((b + np.uint32(0x7FF) + lsb) & np.uint32(0xFFFFF000)).view(np.float32)


def build(nshard=1024, nregions=R, nbatch=B):
    """Build the per-core SPMD graph. All cores run the same NEFF."""
    NT = nshard // 512  # 512-wide n-tiles per region
    BRC = (nregions * nbatch + 127) // 128  # 128-wide br chunks (br = r*32+b)
    BRW = BRC * 128

    nc = bacc.Bacc("TRN2", target_bir_lowering=False, debug=False)

    emb = nc.dram_tensor("emb", [nregions, H, nshard], F32R, kind="ExternalInput").ap()
    wt = nc.dram_tensor("wt", [H, P], F32R, kind="ExternalInput").ap()
    wtT = nc.dram_tensor("wtT", [P, H], F32R, kind="ExternalInput").ap()
    wv = nc.dram_tensor("wv", [H, P], F32R, kind="ExternalInput").ap()
    rfT = nc.dram_tensor("rfT", [H, BRW], F32R, kind="ExternalInput").ap()
    bvrow = nc.dram_tensor("bvrow", [1, P], F32R, kind="ExternalInput").ap()
    btp = nc.dram_tensor("btp", [128, NP], F32, kind="ExternalInput").ap()
    btc = nc.dram_tensor("btc", [128, NP], F32R, kind="ExternalInput").ap()
    onesf_d = nc.dram_tensor("onesf", [128, 128], F32R, kind="ExternalInput").ap()
    onesr_d = nc.dram_tensor("onesr", [1, 512], F32R, kind="ExternalInput").ap()
    identf_d = nc.dram_tensor("identf", [128, 128], F32R, kind="ExternalInput").ap()
    mask_d = nc.dram_tensor(
        "maskrow", [nregions, nshard], F32R, kind="ExternalInput"
    ).ap()

    out_sim = nc.dram_tensor("out_sim", [nbatch, nregions], F32, kind="ExternalOutput").ap()
    out_idx = nc.dram_tensor("out_idx", [nbatch, nregions], U32, kind="ExternalOutput").ap()

    with tile.TileContext(nc) as tc:
        with (
            tc.tile_pool(name="const", bufs=1) as constp,
            tc.tile_pool(name="wpool", bufs=1) as wpool,
        ):
            # ---------- constants / weights ----------
            wt_sb = [constp.tile([128, P], F32R, tag=f"wt{h}", name=f"wt_sb{h}") for h in range(NH)]
            for h in range(NH):
                nc.sync.dma_start(wt_sb[h][:], wt[h * 128 : (h + 1) * 128, :])
            btp_sb = constp.tile([128, NP], F32, tag="btp")
            nc.sync.dma_start(btp_sb[:], btp)
            onesf = constp.tile([128, 128], F32R, tag="onesf")
            nc.sync.dma_start(onesf[:], onesf_d)
            onesr = constp.tile([1, 512], F32R, tag="onesr")
            nc.sync.dma_start(onesr[:], onesr_d)


            # ---------- visual path ----------
            w_sb = [wpool.tile([128, BRW], F32R, tag=f"w{h}", name=f"w_sb{h}") for h in range(NH)]
            c_sb = wpool.tile([1, BRW], F32R, tag="c")
            with (
                tc.tile_pool(name="vps", bufs=2, space="PSUM") as vps,
                tc.tile_pool(name="vsb", bufs=2) as vsb,
                tc.tile_pool(name="vconst", bufs=1) as vconst,
                tc.tile_pool(name="vtps", bufs=2, space="PSUM") as vtps,
            ):
                wtT_sb = [vconst.tile([128, H], F32R, tag=f"wtT{p}", name=f"wtT_sb{p}") for p in range(NP)]
                for p in range(NP):
                    nc.sync.dma_start(wtT_sb[p][:], wtT[p * 128 : (p + 1) * 128, :])
                wv_sb = [vconst.tile([128, P], F32R, tag=f"wv{h}", name=f"wv_sb{h}") for h in range(NH)]
                for h in range(NH):
                    nc.sync.dma_start(wv_sb[h][:], wv[h * 128 : (h + 1) * 128, :])
                rfT_sb = [vconst.tile([128, BRW], F32R, tag=f"rfT{h}", name=f"rfT_sb{h}") for h in range(NH)]
                for h in range(NH):
                    nc.sync.dma_start(rfT_sb[h][:], rfT[h * 128 : (h + 1) * 128, :])
                bv_sb = vconst.tile([1, P], F32R, tag="bv")
                nc.sync.dma_start(bv_sb[:], bvrow)
                btc_sb = vconst.tile([128, NP], F32R, tag="btc")
                nc.sync.dma_start(btc_sb[:], btc)
                identf = vconst.tile([128, 128], F32R, tag="identf")
                nc.sync.dma_start(identf[:], identf_d)
                vn_sb = []
                for bc in range(BRC):
                    mv_ps = vps.tile([128, P], F32, tag="mv")
                    for h in range(NH):
                        nc.tensor.matmul(
                            mv_ps[:],
                            rfT_sb[h][:, bc * 128 : (bc + 1) * 128],
                            wv_sb[h][:],
                            start=(h == 0),
                            stop=False,
                        )
                    nc.tensor.matmul(
                        mv_ps[:],
                        onesr[0:1, 0:128],
                        bv_sb[:],
                        start=False,
                        stop=True,
                    )
                    # qv = sum_p mv^2 per br
                    sq_scratch = vsb.tile([128, P], F32, tag="sqs")
                    qv = vsb.tile([128, 1], F32, tag="qv")
                    nc.scalar.activation(
                        sq_scratch[:],
                        mv_ps[:],
                        mybir.ActivationFunctionType.Square,
                        bias=0.0,
                        scale=1.0,
                        accum_out=qv[:],
                    )
                    # rv = rsqrt(qv) with one Newton step
                    qi = vsb.tile([128, 1], F32, tag="qi")
                    nc.vector.reciprocal(qi[:], qv[:])
                    r0 = vsb.tile([128, 1], F32, tag="r0")
                    nc.scalar.sqrt(r0[:], qi[:])
                    t1 = vsb.tile([128, 1], F32, tag="t1")
                    nc.vector.tensor_mul(t1[:], r0[:], r0[:])
                    nc.vector.tensor_mul(t1[:], t1[:], qv[:])
                    nc.scalar.activation(
                        t1[:],
                        t1[:],
                        mybir.ActivationFunctionType.Copy,
                        bias=1.5,
                        scale=-0.5,
                    )
                    rv = vsb.tile([128, 1], F32, tag="rv")
                    nc.vector.tensor_mul(rv[:], r0[:], t1[:])
                    # vn = mv * rv  (fp32r out)
                    vn_t = vsb.tile([128, P], F32R, tag="vn")
                    nc.scalar.activation(
                        vn_t[:],
                        mv_ps[:],
                        mybir.ActivationFunctionType.Copy,
                        bias=0.0,
                        scale=rv[:],
                    )
                    vn_sb.append((bc, vn_t))

                # vnT via PE transpose
                vnT_sb = [vconst.tile([128, BRW], F32R, tag=f"vnT{p}", name=f"vnT_sb{p}") for p in range(NP)]
                for bc, vn_t in vn_sb:
                    for p in range(NP):
                        tp = vtps.tile([128, 128], F32R, tag="vt")
                        nc.tensor.transpose(
                            tp[:], vn_t[:, p * 128 : (p + 1) * 128], identf[:]
                        )
                        nc.scalar.copy(
                            vnT_sb[p][:, bc * 128 : (bc + 1) * 128],
                            tp[:].bitcast(F32),
                        )

                # w[h, br] = sum_p WtT[p,h] * vnT[p,br]  (fp32r out)
                spans = [(o, min(512, BRW - o)) for o in range(0, BRW, 512)]
                for h in range(NH):
                    for off, wdt in spans:
                        w_ps = vps.tile([128, 512], F32, tag="wps")
                        for p in range(NP):
                            nc.tensor.matmul(
                                w_ps[:, 0:wdt],
                                wtT_sb[p][:, h * 128 : (h + 1) * 128],
                                vnT_sb[p][:, off : off + wdt],
                                start=(p == 0),
                                stop=(p == NP - 1),
                            )
                        nc.scalar.copy(
                            w_sb[h][:, off : off + wdt],
                            w_ps[:, 0:wdt].bitcast(F32),
                        )
                # c[br] = sum_p btc[p] * vnT[p, br]
                for off, wdt in spans:
                    c_ps = vps.tile([1, 512], F32, tag="cps")
                    for p in range(NP):
                        nc.tensor.matmul(
                            c_ps[:, 0:wdt],
                            btc_sb[:, p : p + 1],
                            vnT_sb[p][:, off : off + wdt],
                            start=(p == 0),
                            stop=(p == NP - 1),
                        )
                    nc.scalar.copy(
                        c_sb[0:1, off : off + wdt],
                        c_ps[:, 0:wdt].bitcast(F32),
                    )

            # ---------- main loop over regions ----------
            outsim_sb = wpool.tile([nbatch, nregions], F32, tag="osim")
            outidx_sb = wpool.tile([nbatch, nregions], U32, tag="oidx")
            with (
                tc.tile_pool(name="embp", bufs=3 * NH * NT) as embp,
                tc.tile_pool(name="zps", bufs=4, space="PSUM") as zps,
                tc.tile_pool(name="zsqp", bufs=12) as zsqp,
                tc.tile_pool(name="qps", bufs=2, space="PSUM") as qps,
                tc.tile_pool(name="nps", bufs=2, space="PSUM") as nps,
                tc.tile_pool(name="spool", bufs=2) as spool,
                tc.tile_pool(name="qsb", bufs=2 * NT) as qsbp,
            ):
                for r in range(nregions):
                    m_t = spool.tile([1, nshard], F32R, tag="mrow", name=f"m_{r}")
                    nc.sync.dma_start(m_t[:], mask_d[r : r + 1, :])
                    embT = {}
                    for h in range(NH):
                        for t in range(NT):
                            e_t = embp.tile([128, 512], F32R, tag="emb")
                            nc.sync.dma_start(
                                e_t[:],
                                emb[r, h * 128 : (h + 1) * 128, t * 512 : (t + 1) * 512],
                            )
                            embT[h, t] = e_t

                    # z + zsq
                    zsq = {}
                    for p in range(NP):
                        zt = [zps.tile([128, 512], F32, tag="z", name=f"z_{r}_{p}_{_t}") for _t in range(NT)]
                        for h in range(NH):
                            for t in range(NT):
                                nc.tensor.matmul(
                                    zt[t][:],
                                    wt_sb[h][:, p * 128 : (p + 1) * 128],
                                    embT[h, t][:],
                                    start=(h == 0),
                                    stop=(h == NH - 1),
                                )
                        for t in range(NT):
                            zq = zsqp.tile([128, 512], F32R, tag="zsq")
                            nc.scalar.activation(
                                zq[:],
                                zt[t][:],
                                mybir.ActivationFunctionType.Square,
                                bias=btp_sb[:, p : p + 1],
                                scale=1.0,
                            )
                            zsq[p, t] = zq

                    # q-reduction + rsqrt chain (rows 0..31 used)
                    rsq = []
                    for t in range(NT):
                        q_ps = qps.tile([128, 512], F32, tag="q")
                        for p in range(NP):
                            nc.tensor.matmul(
                                q_ps[:],
                                onesf[:],
                                zsq[p, t][:],
                                start=(p == 0),
                                stop=(p == NP - 1),
                            )
                        qi = qsbp.tile([nbatch, 512], F32, tag="qi")
                        qscr = qsbp.tile([nbatch, 512], F32, tag="qscr")
                        nc.vector.reciprocal_approx_accurate(
                            qi[:], q_ps[0:nbatch, :], qscr[:]
                        )
                        r0 = qsbp.tile([nbatch, 512], F32, tag="r0")
                        nc.scalar.sqrt(r0[:], qi[:])
                        # Newton: r1 = r0*(1.5 - 0.5*q*r0^2)
                        t1 = qsbp.tile([nbatch, 512], F32, tag="t1")
                        nc.vector.tensor_mul(t1[:], r0[:], r0[:])
                        nc.vector.tensor_mul(t1[:], t1[:], q_ps[0:nbatch, :])
                        nc.scalar.activation(
                            t1[:],
                            t1[:],
                            mybir.ActivationFunctionType.Copy,
                            bias=1.5,
                            scale=-0.5,
                        )
                        r1 = qsbp.tile([nbatch, 512], F32, tag="r1")
                        nc.vector.tensor_mul(r1[:], r0[:], t1[:])
                        rsq.append(r1)

                    # num + sims
                    sims = spool.tile([nbatch, nshard], F32, tag="sims")
                    for t in range(NT):
                        n_ps = nps.tile([nbatch, 512], F32, tag="num")
                        for h in range(NH):
                            nc.tensor.matmul(
                                n_ps[:],
                                w_sb[h][:, r * nbatch : (r + 1) * nbatch],
                                embT[h, t][:],
                                start=(h == 0),
                                stop=False,
                            )
                        nc.tensor.matmul(
                            n_ps[:],
                            c_sb[0:1, r * nbatch : (r + 1) * nbatch],
                            onesr[:],
                            start=False,
                            stop=False,
                        )
                        nc.tensor.matmul(
                            n_ps[:],
                            onesr[0:1, 0:nbatch],
                            m_t[0:1, t * 512 : (t + 1) * 512],
                            start=False,
                            stop=True,
                        )
                        nc.vector.tensor_mul(
                            sims[:, t * 512 : (t + 1) * 512], n_ps[:], rsq[t][:]
                        )

                    # top-1
                    top8 = spool.tile([nbatch, 8], F32, tag="top8")
                    idx8 = spool.tile([nbatch, 8], U32, tag="idx8")
                    nc.vector.max(top8[:], sims[:])
                    nc.vector.max_index(idx8[:], top8[:], sims[:])
                    nc.scalar.copy(outsim_sb[:, r : r + 1], top8[:, 0:1])
                    nc.vector.tensor_copy(outidx_sb[:, r : r + 1], idx8[:, 0:1])

                nc.sync.dma_start(out_sim, outsim_sb[:])
                nc.sync.dma_start(out_idx, outidx_sb[:])

    nc.compile()
    return nc


_CACHE = {}


def _get_compiled(nshard, nregions, nbatch):
    key = (nshard, nregions, nbatch)
    if key not in _CACHE:
        _CACHE[key] = build(nshard, nregions, nbatch)
    return _CACHE[key]


def prep_inputs(visual_features, text_embeddings, Wv, bv, Wt, bt, valid_counts,
                nshard=1024):
    """Host-side layout prep + fp32r pre-rounding. Returns per-core in_maps."""
    vf = np.asarray(visual_features, np.float32)
    te = np.asarray(text_embeddings, np.float32)
    Wv = np.asarray(Wv, np.float32)
    bv = np.asarray(bv, np.float32)
    Wt = np.asarray(Wt, np.float32)
    bt = np.asarray(bt, np.float32)
    vc = np.asarray(valid_counts, np.int64)

    nregions, N = te.shape[0], te.shape[1]
    nbatch = vf.shape[0]
    BRC = (nregions * nbatch + 127) // 128
    BRW = BRC * 128

    rf = vf[:, 1 : 1 + nregions, :]  # [B, R, H]
    rfT = np.ones((H, BRW), np.float32)
    rfT[:, : nregions * nbatch] = rf.transpose(2, 1, 0).reshape(H, -1)

    btp = np.ascontiguousarray(bt.reshape(NP, 128).T)

    common = {
        "wt": rne12(Wt),
        "wtT": rne12(np.ascontiguousarray(Wt.T)),
        "wv": rne12(Wv),
        "rfT": rne12(rfT),
        "bvrow": rne12(bv[None, :]),
        "btp": btp,
        "btc": rne12(btp),
        "onesf": np.ones((128, 128), np.float32),
        "onesr": np.ones((1, 512), np.float32),
        "identf": np.eye(128, dtype=np.float32),
    }

    teT = te.transpose(0, 2, 1)  # [R, H, N] view
    in_maps = []
    for c in range(NCORES):
        sl = slice(c * nshard, (c + 1) * nshard)
        m = dict(common)
        m["emb"] = rne12(teT[:, :, sl])
        nidx = np.arange(c * nshard, (c + 1) * nshard)
        mask = np.where(nidx[None, :] < vc[:, None], 0.0, MASKVAL).astype(np.float32)
        m["maskrow"] = mask
        in_maps.append(m)
    return in_maps


def combine(results, nshard=1024):
    """Host-side combine of per-core partial results."""
    sims8 = np.stack([r["out_sim"] for r in results])  # [8, B, R]
    idx8 = np.stack([r["out_idx"] for r in results]).astype(np.int64)
    gc = np.argmax(sims8, axis=0)  # [B, R]; first max wins ties
    best_sim = np.take_along_axis(sims8, gc[None], 0)[0]
    local = np.take_along_axis(idx8, gc[None], 0)[0]
    best_idx = (local + nshard * gc).astype(np.int32)
    return best_sim.astype(np.float32), best_idx


def kernel(visual_features, text_embeddings, Wv, bv, Wt, bt, valid_counts):
    nshard = text_embeddings.shape[1] // NCORES
    in_maps = prep_inputs(
        visual_features, text_embeddings, Wv, bv, Wt, bt, valid_counts, nshard
    )
    nc = _get_compiled(nshard, text_embeddings.shape[0], visual_features.shape[0])
    res = run_bass_kernel_spmd(nc, in_maps, core_ids=list(range(NCORES)))
    return combine(res.results, nshard)


# revision 9
# speedup vs baseline: 1.1007x; 1.0334x over previous
"""Trainium2 Bass kernel for nn_AnatomicalTextEnhancer (retrieval_knn).

Data-parallel over the phrase axis N: each of the 8 NeuronCores scores
all 29 regions x 32 batch queries against its 1024-phrase shard of the
text table; the host combines the 8 per-core (top-sim, top-idx) pairs.

Numerics: every tensor entering the TensorEngine is rounded once to
fp32r (RNE at the 11-bit-mantissa boundary, matching the HW DMA cast);
all downstream matmul arithmetic is then exact (f32 PSUM accumulation),
so the device computes the exact similarity of a once-perturbed dataset.
This was verified host-side to reproduce the f32 reference argmax with
zero flips and best-sim rel-err ~6e-5.
"""

import os
import sys
import tempfile

import numpy as np

sys.path.insert(0, "/opt/trn_rl_repo")
sys.path.insert(0, "/opt/pypackages")

import concourse.bass as bass
import concourse.mybir as mybir
import concourse.tile as tile
from concourse import bacc
from concourse.bass_utils import run_bass_kernel_spmd

F32 = mybir.dt.float32
F32R = mybir.dt.float32r
U32 = mybir.dt.uint32

R, B, H, P = 29, 32, 768, 512
NH = H // 128  # 6 h-chunks
NP = P // 128  # 4 p-chunks
NCORES = 8
MASKVAL = -float(2**30)


def rne12(x):
    """Round f32 -> fp32r (11-bit explicit mantissa), RNE. Matches TRN2."""
    b = np.ascontiguousarray(np.asarray(x, np.float32)).view(np.uint32)
    lsb = (b >> np.uint32(12)) & np.uint32(1)
    return ((b + np.uint32(0x7FF) + lsb) & np.uint32(0xFFFFF000)).view(np.float32)


def build(nshard=1024, nregions=R, nbatch=B):
    """Build the per-core SPMD graph. All cores run the same NEFF."""
    NT = nshard // 512  # 512-wide n-tiles per region
    BRC = (nregions * nbatch + 127) // 128  # 128-wide br chunks (br = r*32+b)
    BRW = BRC * 128

    nc = bacc.Bacc("TRN2", target_bir_lowering=False, debug=False)

    emb = nc.dram_tensor("emb", [nregions, H, nshard], F32R, kind="ExternalInput").ap()
    wt = nc.dram_tensor("wt", [H, P], F32R, kind="ExternalInput").ap()
    wtT = nc.dram_tensor("wtT", [P, H], F32R, kind="ExternalInput").ap()
    wv = nc.dram_tensor("wv", [H, P], F32R, kind="ExternalInput").ap()
    rfT = nc.dram_tensor("rfT", [H, BRW], F32R, kind="ExternalInput").ap()
    bvrow = nc.dram_tensor("bvrow", [1, P], F32R, kind="ExternalInput").ap()
    btp = nc.dram_tensor("btp", [128, NP], F32, kind="ExternalInput").ap()
    btc = nc.dram_tensor("btc", [128, 2 * NP], F32R, kind="ExternalInput").ap()
    onesf_d = nc.dram_tensor("onesf", [128, 128], F32R, kind="ExternalInput").ap()
    onesr_d = nc.dram_tensor("onesr", [1, 1024], F32R, kind="ExternalInput").ap()
    identf_d = nc.dram_tensor("identf", [128, 128], F32R, kind="ExternalInput").ap()
    mask_d = nc.dram_tensor(
        "maskrow", [2 * nregions, nshard], F32R, kind="ExternalInput"
    ).ap()
    e2_d = nc.dram_tensor("e2row", [1, 2], F32R, kind="ExternalInput").ap()

    out_sim = nc.dram_tensor("out_sim", [nbatch, nregions], F32, kind="ExternalOutput").ap()
    out_idx = nc.dram_tensor("out_idx", [nbatch, nregions], U32, kind="ExternalOutput").ap()

    with tile.TileContext(nc) as tc:
        with (
            tc.tile_pool(name="const", bufs=1) as constp,
            tc.tile_pool(name="wpool", bufs=1) as wpool,
        ):
            # ---------- constants / weights ----------
            wt_sb = [constp.tile([128, P], F32R, tag=f"wt{h}", name=f"wt_sb{h}") for h in range(NH)]
            btp_sb = constp.tile([128, NP], F32, tag="btp")
            nc.sync.dma_start(btp_sb[:], btp)
            onesf = constp.tile([128, 128], F32R, tag="onesf")
            nc.sync.dma_start(onesf[:], onesf_d)
            onesr = constp.tile([1, 1024], F32R, tag="onesr")
            nc.sync.dma_start(onesr[:], onesr_d)


            # ---------- visual path ----------
            w_sb = [wpool.tile([128, BRW], F32R, tag=f"w{h}", name=f"w_sb{h}") for h in range(NH)]
            cm_sb = wpool.tile([2, BRW], F32R, tag="cm")
            with (
                tc.tile_pool(name="vps", bufs=2, space="PSUM") as vps,
                tc.tile_pool(name="vsb", bufs=2) as vsb,
                tc.tile_pool(name="vconst", bufs=1) as vconst,
                tc.tile_pool(name="vtps", bufs=2, space="PSUM") as vtps,
            ):
                rfT_sb = [vconst.tile([128, BRW], F32R, tag=f"rfT{h}", name=f"rfT_sb{h}") for h in range(NH)]
                wv_sb = [vconst.tile([128, P], F32R, tag=f"wv{h}", name=f"wv_sb{h}") for h in range(NH)]
                for h in range(NH):
                    nc.sync.dma_start(rfT_sb[h][:], rfT[h * 128 : (h + 1) * 128, :])
                    nc.sync.dma_start(wv_sb[h][:], wv[h * 128 : (h + 1) * 128, :])
                wtT_sb = [vconst.tile([128, H], F32R, tag=f"wtT{p}", name=f"wtT_sb{p}") for p in range(NP)]
                for p in range(NP):
                    nc.sync.dma_start(wtT_sb[p][:], wtT[p * 128 : (p + 1) * 128, :])
                bv_sb = vconst.tile([1, P], F32R, tag="bv")
                nc.sync.dma_start(bv_sb[:], bvrow)
                btc_sb = vconst.tile([128, 2 * NP], F32R, tag="btc")
                nc.sync.dma_start(btc_sb[:], btc)
                e2_sb = vconst.tile([1, 2], F32R, tag="e2")
                nc.sync.dma_start(e2_sb[:], e2_d)
                identf = vconst.tile([128, 128], F32R, tag="identf")
                nc.sync.dma_start(identf[:], identf_d)
                for h in range(NH):
                    nc.sync.dma_start(wt_sb[h][:], wt[h * 128 : (h + 1) * 128, :])
                vn_sb = []
                for bc in range(BRC):
                    mv_ps = vps.tile([128, P], F32, tag="mv")
                    for h in range(NH):
                        nc.tensor.matmul(
                            mv_ps[:],
                            rfT_sb[h][:, bc * 128 : (bc + 1) * 128],
                            wv_sb[h][:],
                            start=(h == 0),
                            stop=False,
                        )
                    nc.tensor.matmul(
                        mv_ps[:],
                        onesr[0:1, 0:128],
                        bv_sb[:],
                        start=False,
                        stop=True,
                    )
                    # qv = sum_p mv^2 per br
                    sq_scratch = vsb.tile([128, P], F32, tag="sqs")
                    qv = vsb.tile([128, 1], F32, tag="qv")
                    nc.scalar.activation(
                        sq_scratch[:],
                        mv_ps[:],
                        mybir.ActivationFunctionType.Square,
                        bias=0.0,
                        scale=1.0,
                        accum_out=qv[:],
                    )
                    # rv = rsqrt(qv) with one Newton step
                    qi = vsb.tile([128, 1], F32, tag="qi")
                    nc.vector.reciprocal(qi[:], qv[:])
                    r0 = vsb.tile([128, 1], F32, tag="r0")
                    nc.scalar.sqrt(r0[:], qi[:])
                    t1 = vsb.tile([128, 1], F32, tag="t1")
                    nc.vector.tensor_mul(t1[:], r0[:], r0[:])
                    nc.vector.tensor_mul(t1[:], t1[:], qv[:])
                    nc.scalar.activation(
                        t1[:],
                        t1[:],
                        mybir.ActivationFunctionType.Copy,
                        bias=1.5,
                        scale=-0.5,
                    )
                    rv = vsb.tile([128, 1], F32, tag="rv")
                    nc.vector.tensor_mul(rv[:], r0[:], t1[:])
                    # vn = mv * rv  (fp32r out)
                    vn_t = vsb.tile([128, P], F32R, tag="vn")
                    nc.scalar.activation(
                        vn_t[:],
                        mv_ps[:],
                        mybir.ActivationFunctionType.Copy,
                        bias=0.0,
                        scale=rv[:],
                    )
                    vn_sb.append((bc, vn_t))

                # vnT via PE transpose
                vnT_sb = [vconst.tile([128, BRW], F32R, tag=f"vnT{p}", name=f"vnT_sb{p}") for p in range(NP)]
                for bc, vn_t in vn_sb:
                    for p in range(NP):
                        tp = vtps.tile([128, 128], F32R, tag="vt")
                        nc.tensor.transpose(
                            tp[:], vn_t[:, p * 128 : (p + 1) * 128], identf[:]
                        )
                        nc.scalar.copy(
                            vnT_sb[p][:, bc * 128 : (bc + 1) * 128],
                            tp[:].bitcast(F32),
                        )

                # w[h, br] = sum_p WtT[p,h] * vnT[p,br]  (fp32r out)
                spans = [(o, min(512, BRW - o)) for o in range(0, BRW, 512)]
                for h in range(NH):
                    for off, wdt in spans:
                        w_ps = vps.tile([128, 512], F32, tag="wps")
                        for p in range(NP):
                            nc.tensor.matmul(
                                w_ps[:, 0:wdt],
                                wtT_sb[p][:, h * 128 : (h + 1) * 128],
                                vnT_sb[p][:, off : off + wdt],
                                start=(p == 0),
                                stop=(p == NP - 1),
                            )
                        nc.scalar.copy(
                            w_sb[h][:, off : off + wdt],
                            w_ps[:, 0:wdt].bitcast(F32),
                        )
                # c[br] = sum_p btc[p] * vnT[p, br]
                for off, wdt in spans:
                    c_ps = vps.tile([2, 512], F32, tag="cps")
                    for p in range(NP):
                        nc.tensor.matmul(
                            c_ps[:, 0:wdt],
                            btc_sb[:, 2 * p : 2 * p + 2],
                            vnT_sb[p][:, off : off + wdt],
                            start=(p == 0),
                            stop=False,
                        )
                    nc.tensor.matmul(
                        c_ps[:, 0:wdt],
                        e2_sb[:],
                        onesr[0:1, 0:wdt],
                        start=False,
                        stop=True,
                    )
                    nc.scalar.copy(
                        cm_sb[0:2, off : off + wdt],
                        c_ps[:, 0:wdt].bitcast(F32),
                    )

            # ---------- main loop over regions ----------
            outsim_sb = wpool.tile([nbatch, nregions], F32, tag="osim")
            outidx_sb = wpool.tile([nbatch, nregions], U32, tag="oidx")
            with (
                tc.tile_pool(name="embp", bufs=3 * NH * NT) as embp,
                tc.tile_pool(name="zps", bufs=4, space="PSUM") as zps,
                tc.tile_pool(name="zsqp", bufs=12) as zsqp,
                tc.tile_pool(name="qps", bufs=2, space="PSUM") as qps,
                tc.tile_pool(name="nps", bufs=2, space="PSUM") as nps,
                tc.tile_pool(name="spool", bufs=2) as spool,
                tc.tile_pool(name="qsb", bufs=2 * NT) as qsbp,
            ):
                for r in range(nregions):
                    m_t = spool.tile([2, nshard], F32R, tag="mrow", name=f"m_{r}")
                    nc.sync.dma_start(m_t[:], mask_d[2 * r : 2 * r + 2, :])
                    embT = {}
                    for h in range(NH):
                        for t in range(NT):
                            e_t = embp.tile([128, 512], F32R, tag="emb")
                            nc.sync.dma_start(
                                e_t[:],
                                emb[r, h * 128 : (h + 1) * 128, t * 512 : (t + 1) * 512],
                            )
                            embT[h, t] = e_t

                    # z + zsq
                    zsq = {}
                    for p in range(NP):
                        zt = [zps.tile([128, 512], F32, tag="z", name=f"z_{r}_{p}_{_t}") for _t in range(NT)]
                        for h in range(NH):
                            for t in range(NT):
                                nc.tensor.matmul(
                                    zt[t][:],
                                    wt_sb[h][:, p * 128 : (p + 1) * 128],
                                    embT[h, t][:],
                                    start=(h == 0),
                                    stop=(h == NH - 1),
                                )
                        for t in range(NT):
                            zq = zsqp.tile([128, 512], F32R, tag="zsq")
                            nc.scalar.activation(
                                zq[:],
                                zt[t][:],
                                mybir.ActivationFunctionType.Square,
                                bias=btp_sb[:, p : p + 1],
                                scale=1.0,
                            )
                            zsq[p, t] = zq

                    # q-reduction + rsqrt chain (rows 0..31 used)
                    rsq = []
                    for t in range(NT):
                        q_ps = qps.tile([128, 512], F32, tag="q")
                        for p in range(NP):
                            nc.tensor.matmul(
                                q_ps[:],
                                onesf[:],
                                zsq[p, t][:],
                                start=(p == 0),
                                stop=(p == NP - 1),
                            )
                        qi = qsbp.tile([nbatch, 512], F32, tag="qi")
                        qscr = qsbp.tile([nbatch, 512], F32, tag="qscr")
                        nc.vector.reciprocal_approx_accurate(
                            qi[:], q_ps[0:nbatch, :], qscr[:]
                        )
                        r0 = qsbp.tile([nbatch, 512], F32, tag="r0")
                        nc.scalar.sqrt(r0[:], qi[:])
                        # Newton: r1 = r0*(1.5 - 0.5*q*r0^2)
                        t1 = qsbp.tile([nbatch, 512], F32, tag="t1")
                        nc.vector.tensor_mul(t1[:], r0[:], r0[:])
                        nc.vector.tensor_mul(t1[:], t1[:], q_ps[0:nbatch, :])
                        nc.scalar.activation(
                            t1[:],
                            t1[:],
                            mybir.ActivationFunctionType.Copy,
                            bias=1.5,
                            scale=-0.5,
                        )
                        r1 = qsbp.tile([nbatch, 512], F32, tag="r1")
                        nc.vector.tensor_mul(r1[:], r0[:], t1[:])
                        rsq.append(r1)

                    # num + sims
                    sims = spool.tile([nbatch, nshard], F32, tag="sims")
                    for t in range(NT):
                        n_ps = nps.tile([nbatch, 512], F32, tag="num")
                        for h in range(NH):
                            nc.tensor.matmul(
                                n_ps[:],
                                w_sb[h][:, r * nbatch : (r + 1) * nbatch],
                                embT[h, t][:],
                                start=(h == 0),
                                stop=False,
                            )
                        nc.tensor.matmul(
                            n_ps[:],
                            cm_sb[0:2, r * nbatch : (r + 1) * nbatch],
                            m_t[0:2, t * 512 : (t + 1) * 512],
                            start=False,
                            stop=True,
                        )
                        nc.vector.tensor_mul(
                            sims[:, t * 512 : (t + 1) * 512], n_ps[:], rsq[t][:]
                        )

                    # top-1
                    top8 = spool.tile([nbatch, 8], F32, tag="top8")
                    idx8 = spool.tile([nbatch, 8], U32, tag="idx8")
                    nc.vector.max(top8[:], sims[:])
                    nc.vector.max_index(idx8[:], top8[:], sims[:])
                    nc.scalar.copy(outsim_sb[:, r : r + 1], top8[:, 0:1])
                    nc.vector.tensor_copy(outidx_sb[:, r : r + 1], idx8[:, 0:1])

                nc.sync.dma_start(out_sim, outsim_sb[:])
                nc.sync.dma_start(out_idx, outidx_sb[:])

    nc.compile()
    return nc


_CACHE = {}


def _get_compiled(nshard, nregions, nbatch):
    key = (nshard, nregions, nbatch)
    if key not in _CACHE:
        _CACHE[key] = build(nshard, nregions, nbatch)
    return _CACHE[key]


def prep_inputs(visual_features, text_embeddings, Wv, bv, Wt, bt, valid_counts,
                nshard=1024):
    """Host-side layout prep + fp32r pre-rounding. Returns per-core in_maps."""
    vf = np.asarray(visual_features, np.float32)
    te = np.asarray(text_embeddings, np.float32)
    Wv = np.asarray(Wv, np.float32)
    bv = np.asarray(bv, np.float32)
    Wt = np.asarray(Wt, np.float32)
    bt = np.asarray(bt, np.float32)
    vc = np.asarray(valid_counts, np.int64)

    nregions, N = te.shape[0], te.shape[1]
    nbatch = vf.shape[0]
    BRC = (nregions * nbatch + 127) // 128
    BRW = BRC * 128

    rf = vf[:, 1 : 1 + nregions, :]  # [B, R, H]
    rfT = np.ones((H, BRW), np.float32)
    rfT[:, : nregions * nbatch] = rf.transpose(2, 1, 0).reshape(H, -1)

    btp = np.ascontiguousarray(bt.reshape(NP, 128).T)

    common = {
        "e2row": np.array([[0.0, 1.0]], np.float32),
        "wt": rne12(Wt),
        "wtT": rne12(np.ascontiguousarray(Wt.T)),
        "wv": rne12(Wv),
        "rfT": rne12(rfT),
        "bvrow": rne12(bv[None, :]),
        "btp": btp,
        "btc": rne12(np.stack([btp, np.zeros_like(btp)], axis=-1).reshape(128, -1)),
        "onesf": np.ones((128, 128), np.float32),
        "onesr": np.ones((1, 1024), np.float32),
        "identf": np.eye(128, dtype=np.float32),
    }

    teT = te.transpose(0, 2, 1)  # [R, H, N] view
    in_maps = []
    for c in range(NCORES):
        sl = slice(c * nshard, (c + 1) * nshard)
        m = dict(common)
        m["emb"] = rne12(teT[:, :, sl])
        nidx = np.arange(c * nshard, (c + 1) * nshard)
        mask = np.where(nidx[None, :] < vc[:, None], 0.0, MASKVAL).astype(np.float32)
        om = np.empty((2 * len(vc), nshard), np.float32)
        om[0::2] = 1.0
        om[1::2] = mask
        m["maskrow"] = om
        in_maps.append(m)
    return in_maps


def combine(results, nshard=1024):
    """Host-side combine of per-core partial results."""
    sims8 = np.stack([r["out_sim"] for r in results])  # [8, B, R]
    idx8 = np.stack([r["out_idx"] for r in results]).astype(np.int64)
    gc = np.argmax(sims8, axis=0)  # [B, R]; first max wins ties
    best_sim = np.take_along_axis(sims8, gc[None], 0)[0]
    local = np.take_along_axis(idx8, gc[None], 0)[0]
    best_idx = (local + nshard * gc).astype(np.int32)
    return best_sim.astype(np.float32), best_idx


def kernel(visual_features, text_embeddings, Wv, bv, Wt, bt, valid_counts):
    nshard = text_embeddings.shape[1] // NCORES
    in_maps = prep_inputs(
        visual_features, text_embeddings, Wv, bv, Wt, bt, valid_counts, nshard
    )
    nc = _get_compiled(nshard, text_embeddings.shape[0], visual_features.shape[0])
    res = run_bass_kernel_spmd(nc, in_maps, core_ids=list(range(NCORES)))
    return combine(res.results, nshard)
